# revision 34
# baseline (speedup 1.0000x reference)
"""Strided (stride=1) valid 1D conv on Trainium2, data-parallel over batch.

Problem: x (16, 32, 32768) f32, kernel (1, 32, 32, 3) f32
         -> out (16, 32, 32766) f32  (valid conv, NCH / OIH layout)

Strategy (per core, 2 batches each across 8 cores):
  out[b, co, l] = sum_{ci,k} W[co, ci, k] * x[b, ci, l + k]

  All device I/O is float16 (x, w and out are converted on the host):
  fp16 in / f32-PSUM accumulate / fp16 out gives rel err ~3e-4 vs the
  f32 reference while halving HBM traffic, which is the binding
  resource (~358-430 GB/s per NeuronCore).  Channel count is 32, so 4
  independent L-chunks ("groups") are packed into the 128 SBUF
  partitions: partition (g*32 + ci) holds x[b, ci, base+g*G+j].  A
  block-diagonal [128, 128] weight matrix (4 copies of W_k^T on the
  diagonal) turns the 4-group conv tap into ONE K=128 matmul; the 3
  taps accumulate into one PSUM bank (start/stop flags).

  Tiling: G=2048 cols/group -> TILE_L=8192 output cols/tile, NT=4
  tiles/batch, NJ=4 512-col matmul chunks/tile.  Even global tiles use
  PSUM banks 0-3 (drained by ACT), odd tiles banks 4-7 (drained by
  DVE): a tile's banks were last used two tiles back by the SAME
  engine, so the tensor engine issues ONE (almost always satisfied)
  drain wait per tile instead of one per matmul group.

  The host pre-packs x into the exact SBUF layout and unpacks the
  packed output, so every device DMA is one contiguous stream per
  partition row.  DMA schedule (phased): HBM R-only and W-only streams
  each run ~600-650 GB/s on this part, but fine-grained R/W mixing
  collapses to ~340-390 GB/s, so the SP ring alternates 4.2MB read
  phases (both pairs' ins) with 4.2MB write phases (four 1MB half-pair
  outs).  xt/osb buffers have 2 slots per pair (rep parity) so rep
  r+2's ins reuse rep r's slots without coupling to the lagging write
  phase.  Single ring: splitting ins/outs across the SP and ACT HWDGE
  rings measured slower (354 GB/s), and hard barriers between phases
  cost more in ring idle than they recover.

  Raw Bass (not Tile): walrus codegen in this toolchain embeds at most
  ONE sync wait per Matmult / HWDGE DMACopy, which Tile's auto-generated
  semaphores routinely exceed.  Every cross-engine wait here is an
  explicit standalone wait_ge on the engine's sequencer.

  reps > 1 repeats the whole pipeline in one NEFF (benchmarking only).
"""

import sys

if "/opt/trn_rl_repo" not in sys.path:
    sys.path.insert(0, "/opt/trn_rl_repo")

from contextlib import ExitStack

import numpy as np

import concourse.bass as bass
import concourse.mybir as mybir
from concourse.bass_utils import run_bass_kernel_spmd

# Problem shape (hardcoded; harness contract)
B, C, L = 16, 32, 32768
CO, KT = 32, 3
LOUT = L - KT + 1  # 32766
NCORES = 8
BPC = B // NCORES  # batches per core = 2

# Padded shapes
LP = L + 2  # x padded with 2 trailing zero columns
LOP = L     # output computed padded to 32768 (last 2 cols garbage)

# Tiling
NG = 4              # L-groups packed across the 128 partitions
G = 2048            # columns per group per tile
TILE_L = NG * G     # 8192 output cols per tile
NT = LOP // TILE_L  # tiles per batch = 4
NJ = G // 512       # 512-wide matmul chunks per group = 4
NTILES = BPC * NT   # 8 tiles/core; pair p (= local batch) owns NT tiles
XW = NT * (G + 2)   # xt pair-row width = 8200
OW = NT * G         # osb pair-row width = 8192

_CACHE = {}


def _cp_sem_count(gt: int, j: int) -> int:
    """Drain-engine sem value after copy (gt, j) completes.

    ACT drains even global tiles, DVE odd ones; each engine's sem counts
    its own copies in order.  gt = rep * NTILES + ti.
    """
    return NJ * (gt // 2) + j + 1


def _build_nc(
    reps: int = 1,
    split_ends: bool = True,
    phased: bool = True,
    whole_outs: bool = False,
    macro: bool = False,
):
    """phased=True: 4 xt/osb slots (pair x rep-parity); the SP ring runs
    alternating 4.2MB read and 4.2MB write phases (HBM R/W mixing costs
    ~40% of pure-stream bandwidth; phase-batching recovers part of it).
    phased=False: legacy 2-slot lag-1 interleave (in, out, in, out)."""
    f32 = mybir.dt.float32
    f16 = mybir.dt.float16

    nc = bass.Bass(trn_type="TRN2", target_bir_lowering=False)
    x = nc.dram_tensor("x", [BPC, 128, XW], f16, kind="ExternalInput")
    w = nc.dram_tensor("w", [128, KT * 128], f16, kind="ExternalInput")
    out = nc.dram_tensor("out", [BPC, 128, OW], f16, kind="ExternalOutput")

    HOW = OW // 2  # half-pair out width (2 tiles)
    NSLOT = 2 if phased else 1  # xt slots per pair (rep parity)
    # osb slots: macro-phasing batches two W phases per 2 reps, so the
    # write of rep r lands ~1 rep later and its slot is reused mod 3
    OSLOT = 3 if (phased and macro) else NSLOT

    with ExitStack() as ctx:
        wt = ctx.enter_context(nc.sbuf_tensor("wt", [128, KT * 128], f16))
        xts = [
            [
                ctx.enter_context(
                    nc.sbuf_tensor(f"xt{p}_{s}", [128, XW], f16)
                )
                for s in range(NSLOT)
            ]
            for p in range(BPC)
        ]
        osbs = [
            [
                ctx.enter_context(
                    nc.sbuf_tensor(f"osb{p}_{s}", [128, OW], f16)
                )
                for s in range(OSLOT)
            ]
            for p in range(BPC)
        ]
        # banks 0..NJ-1: even tiles (ACT), NJ..2*NJ-1: odd tiles (DVE)
        psums = [
            ctx.enter_context(nc.psum_tensor(f"ps{j}", [128, 512], f32))
            for j in range(2 * NJ)
        ]
        sem_w = ctx.enter_context(nc.semaphore("sem_w"))
        sem_xs = [
            [
                ctx.enter_context(nc.semaphore(f"sem_x{p}_{s}"))
                for s in range(NSLOT)
            ]
            for p in range(BPC)
        ]
        sem_mm = ctx.enter_context(nc.semaphore("sem_mm"))
        sem_cpa = ctx.enter_context(nc.semaphore("sem_cpa"))
        sem_cpb = ctx.enter_context(nc.semaphore("sem_cpb"))
        # per-(pair, slot, half) out sems (a counting sem shared by
        # concurrently in-flight DMAs is unsound; same-sem DMAs here are
        # strictly ordered by the drain/out guard chain)
        sem_out = [
            [
                [
                    ctx.enter_context(nc.semaphore(f"sem_o{p}_{s}_{h}"))
                    for h in range(2)
                ]
                for s in range(OSLOT)
            ]
            for p in range(BPC)
        ]
        # fragment sems for the split boundary DMAs
        sem_xt = ctx.enter_context(nc.semaphore("sem_xt"))
        sem_xq = ctx.enter_context(nc.semaphore("sem_xq"))
        block = ctx.enter_context(nc.Block())

        def issue_in(sync, p: int, r: int):
            s = r % NSLOT
            if split_ends and r == 0 and p == 0:
                # fill latency: land tile 0's first half (covers matmul
                # chunks j < NJ/2), then the rest of tile 0, then the
                # remaining tiles, so PE starts early in the fill
                H = G + 2
                Q = (NJ // 2 - 1) * 512 + 514
                sync.dma_start(
                    out=xts[p][s][:, 0:Q], in_=x[p, :, 0:Q]
                ).then_inc(sem_xs[p][s], 16)
                sync.dma_start(
                    out=xts[p][s][:, Q:H], in_=x[p, :, Q:H]
                ).then_inc(sem_xq, 16)
                sync.dma_start(
                    out=xts[p][s][:, H:XW], in_=x[p, :, H:XW]
                ).then_inc(sem_xt, 16)
            else:
                sync.dma_start(
                    out=xts[p][s][:], in_=x[p, :, :]
                ).then_inc(sem_xs[p][s], 16)

        def issue_outs(sync, p: int, r: int):
            s = r % OSLOT
            gt0 = r * NTILES + p * NT
            if whole_outs:
                # one 2.1MB out per pair, tracked on the h=0 sem only
                sync.wait_ge(sem_cpa, _cp_sem_count(gt0 + 2, NJ - 1))
                sync.wait_ge(sem_cpb, _cp_sem_count(gt0 + 3, NJ - 1))
                sync.dma_start(
                    out=out[p, :, :], in_=osbs[p][s][:]
                ).then_inc(sem_out[p][s][0], 16)
                return
            sync.wait_ge(sem_cpa, _cp_sem_count(gt0, NJ - 1))
            sync.wait_ge(sem_cpb, _cp_sem_count(gt0 + 1, NJ - 1))
            sync.dma_start(
                out=out[p, :, 0:HOW], in_=osbs[p][s][:, 0:HOW]
            ).then_inc(sem_out[p][s][0], 16)
            sync.wait_ge(sem_cpa, _cp_sem_count(gt0 + 2, NJ - 1))
            sync.wait_ge(sem_cpb, _cp_sem_count(gt0 + 3, NJ - 1))
            sync.dma_start(
                out=out[p, :, HOW:OW], in_=osbs[p][s][:, HOW:OW]
            ).then_inc(sem_out[p][s][1], 16)

        @block.sync
        def _(sync):
            sync.dma_start(out=wt[:], in_=w[:, :]).then_inc(sem_w, 16)
            if phased and macro:
                # Macro schedule: R(0) R(1) | R(2) R(3) W(0) W(1) |
                # R(4) R(5) W(2) W(3) | ... — 8.4MB read and write
                # phases per 2 reps halve the R/W turnaround count and
                # amortize phase gating.  xt slots stay mod-2 (R(q)
                # reuses rep q-2's slot, gated on its matmuls); osb
                # slots are mod-3 because W(r) completes ~1 rep late.
                for q in range(min(2, reps)):
                    for p in range(BPC):
                        issue_in(sync, p, q)
                for r in range(0, reps, 2):
                    for q in (r + 2, r + 3):
                        if q < reps:
                            for p in range(BPC):
                                sync.wait_ge(
                                    sem_mm,
                                    NJ * ((q - 2) * NTILES + (p + 1) * NT),
                                )
                                issue_in(sync, p, q)
                    for q in (r, r + 1):
                        if q < reps:
                            for p in range(BPC):
                                issue_outs(sync, p, q)
            elif phased:
                # Ring schedule: R(0) R(1) | R(2) W(0) | R(3) W(1) | ...
                # Alternating 4.2MB read and 4.2MB write phases; mixing
                # R and W at fine grain costs ~40% of pure-stream HBM
                # bandwidth.  R(r+2) reuses rep r's xt slots, gated on
                # rep r's matmuls; W(r) is gated on rep r's drains.
                for q in range(min(2, reps)):
                    for p in range(BPC):
                        issue_in(sync, p, q)
                for r in range(reps):
                    if r + 2 < reps:
                        for p in range(BPC):
                            sync.wait_ge(
                                sem_mm, NJ * (r * NTILES + (p + 1) * NT)
                            )
                            issue_in(sync, p, r + 2)
                    for p in range(BPC):
                        issue_outs(sync, p, r)
            else:
                # Legacy lag-1 interleave: in(gp), outs(gp-1), ...
                NPAIR = BPC * reps
                for gp in range(NPAIR + 1):
                    if gp < NPAIR:
                        p = gp % BPC
                        r = gp // BPC
                        if r > 0:
                            sync.wait_ge(
                                sem_mm, NJ * ((r - 1) * NTILES + (p + 1) * NT)
                            )
                        issue_in(sync, p, r)
                    op = gp - 1
                    if op >= 0:
                        issue_outs(sync, op % BPC, op // BPC)
            for p in range(BPC):
                for s in range(OSLOT):
                    n_s = len([r for r in range(reps) if r % OSLOT == s])
                    if n_s:
                        for h in range(1 if whole_outs else 2):
                            sync.wait_ge(sem_out[p][s][h], 16 * n_s)

        @block.tensor
        def _(tensor):
            tensor.wait_ge(sem_w, 16)
            for r in range(reps):
                sl = r % NSLOT
                for ti in range(NTILES):
                    gt = r * NTILES + ti
                    p, u = divmod(ti, NT)
                    if split_ends and r == 0 and p == 0:
                        # pair 0 rep 0 arrives in fragments; tile u only
                        # reads its own fragment(s)
                        if u == 0:
                            tensor.wait_ge(sem_xs[p][sl], 16)
                        elif u == 1:
                            tensor.wait_ge(sem_xt, 16)
                    elif u == 0:
                        tensor.wait_ge(sem_xs[p][sl], 16 * (r // NSLOT + 1))
                    if gt >= 2:
                        # this tile's PSUM bank set was drained two
                        # tiles back by the same-parity engine; one wait
                        # covers all NJ banks (copies are FIFO/engine)
                        cur_sem = sem_cpa if gt % 2 == 0 else sem_cpb
                        tensor.wait_ge(cur_sem, NJ * (gt // 2))
                    xbase = u * (G + 2)
                    pbase = (gt % 2) * NJ
                    for j in range(NJ):
                        if split_ends and gt == 0 and j == NJ // 2:
                            # 2nd fragment of the first fill set
                            tensor.wait_ge(sem_xq, 16)
                        mm = None
                        for k in range(KT):
                            a = xbase + j * 512 + k
                            mm = tensor.matmul(
                                psums[pbase + j][:],
                                wt[:, k * 128 : (k + 1) * 128],
                                xts[p][sl][:, a : a + 512],
                                start=(k == 0),
                                stop=(k == KT - 1),
                            )
                        mm.then_inc(sem_mm, 1)

        def drain(eng, copy_fn, parity, cp_sem):
            # engine drains tiles of its parity (bank set parity*NJ..)
            for gt in range(parity, NTILES * reps, 2):
                r, ti = divmod(gt, NTILES)
                sl = r % OSLOT
                p, u = divmod(ti, NT)
                obase = u * G
                for j in range(NJ):
                    eng.wait_ge(sem_mm, gt * NJ + j + 1)
                    if r >= OSLOT and j == 0:
                        # osb half reuse: that half's previous out-DMA
                        # from this slot must have left the building
                        h = 0 if (whole_outs or u < NT // 2) else 1
                        eng.wait_ge(sem_out[p][sl][h], 16 * (r // OSLOT))
                    copy_fn(
                        osbs[p][sl][
                            :, obase + j * 512 : obase + (j + 1) * 512
                        ],
                        psums[parity * NJ + j][:],
                    ).then_inc(cp_sem, 1)

        @block.scalar
        def _(scalar):
            drain(scalar, scalar.copy, 0, sem_cpa)

        @block.vector
        def _(vector):
            drain(vector, vector.tensor_copy, 1, sem_cpb)

    return nc


def _block_diag_weights(kernel: np.ndarray) -> np.ndarray:
    """kernel (1, CO, C, KT) -> (128, KT*128) f16 block-diag lhsT, SBUF layout.

    row (ci + 32*g), col (k*128 + co + 32*g) = kernel[0, co, ci, k]
    """
    wbd = np.zeros((KT, 128, 128), dtype=np.float16)
    wt = np.ascontiguousarray(kernel[0].transpose(2, 1, 0)).astype(np.float16)
    for g in range(NG):
        wbd[:, g * 32 : (g + 1) * 32, g * 32 : (g + 1) * 32] = wt
    return np.ascontiguousarray(wbd.transpose(1, 0, 2)).reshape(128, KT * 128)


def _pack_x(x: np.ndarray) -> np.ndarray:
    """(B, C, L) -> (NCORES, BPC, 128, XW) f16 packed, padded by 2.

    Row (g*32 + ci), col (t*(G+2) + j) of batch b's block holds
    x[b, ci, t*TILE_L + g*G + j] (zeros past L).
    """
    xp = np.zeros((B, C, LP), dtype=np.float16)
    xp[:, :, :L] = x.astype(np.float16)
    sb, sc, sl = (s // 2 for s in xp.strides)
    win = np.lib.stride_tricks.as_strided(
        xp,
        shape=(B, NG, C, NT, G + 2),
        strides=tuple(2 * s for s in (sb, G, sc, TILE_L, sl)),
    )
    return np.ascontiguousarray(win).reshape(NCORES, BPC, 128, XW)


def _unpack_out(packed: np.ndarray) -> np.ndarray:
    """(NCORES, BPC, 128, OW) f16 -> (B, CO, LOUT) f32."""
    arr = packed.reshape(NCORES, BPC, NG, CO, NT, G)
    arr = arr.transpose(0, 1, 3, 4, 2, 5)  # core, b, co, t, g, j
    arr = np.ascontiguousarray(arr).astype(np.float32)
    return arr.reshape(B, CO, LOP)[:, :, :LOUT]


def kernel(x: np.ndarray, kernel: np.ndarray) -> np.ndarray:
    if "nc" not in _CACHE:
        _CACHE["nc"] = _build_nc()
    nc = _CACHE["nc"]

    wbd = _block_diag_weights(np.asarray(kernel, dtype=np.float32))
    xpk = _pack_x(np.asarray(x))

    in_maps = [{"x": xpk[i], "w": wbd} for i in range(NCORES)]
    res = run_bass_kernel_spmd(nc, in_maps, list(range(NCORES)))
    packed = np.stack([r["out"] for r in res.results], axis=0)
    return _unpack_out(packed)
